# revision 1
# baseline (speedup 1.0000x reference)
"""TRN2 Bass kernel for nn_GCNBlock_77927886618861 (gnn_message_passing).

Reference computation (per batch b, K=5 neighbors, H=8192 positions):
    d = diff_patch.reshape(b,5,192,H)
    x = w1@d + b1; r = relu(wr1@x + br1); r = wr2@r + br2; x = x + 0.1*r
    logits = w3@x + b3; wgt = softmax_k(logits)
    knn = sum_k knn_hr_k * wgt_k;  knn_lr = mean-pool(knn, scale=2)

The 1x1-conv MLP collapses algebraically (softmax is invariant to adding a
constant across k, so all constant terms drop):
    logits'_k = a.d_k + v.relu(G.d_k + g0)
    G = wr1@w1 [64,192], g0 = wr1@b1+br1, a = w3@w1 [192], v = 0.1*w3@wr2 [64]

Sharding: one batch (of 8) per NeuronCore. Big matmuls in float32r
(1 cyc/col on the PE). Softmax via exp -> ones-matmul sum -> ln ->
partition-broadcast DMA -> subtract -> exp (no reciprocal). The k-sum of
hr*wgt uses PE fold matmuls (cross-partition adds are illegal on DVE).
"""
import sys

import numpy as np

sys.path.insert(0, "/opt/trn_rl_repo")

import concourse.bass as bass  # noqa: E402
import concourse.bacc as bacc  # noqa: E402
import concourse.tile as tile  # noqa: E402
from concourse import mybir  # noqa: E402
from concourse.bass_utils import run_bass_kernel_spmd  # noqa: E402

F32 = mybir.dt.float32
F32R = mybir.dt.float32r
AF = mybir.ActivationFunctionType
ALU = mybir.AluOpType

B, K, C, H, DN, SCALE = 8, 5, 64, 8192, 192, 2
RES_SCALE = 0.1
NT = 512                 # positions per tile
NTILES = H // NT         # 16
HHALF = H // SCALE       # 4096

_CACHE = {}


def _build_nc():
    nc = bacc.Bacc("TRN2", target_bir_lowering=False, debug=False)

    d_d = nc.dram_tensor("d", [K * DN, H], F32R, kind="ExternalInput")
    hr_d = nc.dram_tensor("hr", [K * C, H], F32, kind="ExternalInput")
    gt_hi_d = nc.dram_tensor("gt_hi", [128, C], F32R, kind="ExternalInput")
    gt_lo_d = nc.dram_tensor("gt_lo", [64, C], F32R, kind="ExternalInput")
    a5_hi_d = nc.dram_tensor("a5_hi", [128, K, K], F32R, kind="ExternalInput")
    a5_lo_d = nc.dram_tensor("a5_lo", [64, K, K], F32R, kind="ExternalInput")
    v5_d = nc.dram_tensor("v5", [C, K, K], F32R, kind="ExternalInput")
    ones5_d = nc.dram_tensor("ones5", [K, 1], F32R, kind="ExternalInput")
    fold2_d = nc.dram_tensor("fold2", [128, C], F32R, kind="ExternalInput")
    i64_d = nc.dram_tensor("i64", [C, C], F32R, kind="ExternalInput")
    g0p_d = nc.dram_tensor("g0p", [C, 1], F32, kind="ExternalInput")

    knn_d = nc.dram_tensor("knn", [C, H], F32, kind="ExternalOutput")
    lr_d = nc.dram_tensor("knn_lr", [C, HHALF], F32, kind="ExternalOutput")

    # [c, k, n] views of the inputs (c = channel within neighbor block)
    d_r = d_d[:].rearrange("(k c) n -> c k n", k=K)     # [192, 5, H]

    with tile.TileContext(nc) as tc:
        with (
            tc.tile_pool(name="wts", bufs=1) as wts,
            tc.tile_pool(name="dbuf", bufs=2) as dbuf,
            tc.tile_pool(name="hrbuf", bufs=2) as hrbuf,
            tc.tile_pool(name="hbuf", bufs=3) as hbuf,
            tc.tile_pool(name="smx", bufs=2) as smx,
            tc.tile_pool(name="wbc", bufs=2) as wbcp,
            tc.tile_pool(name="prod", bufs=2) as prod,
            tc.tile_pool(name="big", bufs=1) as big,
            tc.tile_pool(name="lrp", bufs=2) as lrp,
            tc.tile_pool(name="ph", bufs=2, space="PSUM") as php,
            tc.tile_pool(name="pl", bufs=2, space="PSUM") as plp,
            tc.tile_pool(name="psum_s", bufs=2, space="PSUM") as pssp,
            tc.tile_pool(name="pknn", bufs=2, space="PSUM") as pknnp,
        ):
            # ---- constants (one-time) ----
            gt_hi = wts.tile([128, C], F32R)
            gt_lo = wts.tile([64, C], F32R)
            a5_hi = wts.tile([128, K, K], F32R)
            a5_lo = wts.tile([64, K, K], F32R)
            v5 = wts.tile([C, K, K], F32R)
            ones5 = wts.tile([K, 1], F32R)
            fold2 = wts.tile([128, C], F32R)
            i64 = wts.tile([C, C], F32R)
            g0p = wts.tile([C, 1], F32)
            for t, dd in (
                (gt_hi, gt_hi_d), (gt_lo, gt_lo_d), (a5_hi, a5_hi_d),
                (a5_lo, a5_lo_d), (v5, v5_d), (ones5, ones5_d),
                (fold2, fold2_d), (i64, i64_d), (g0p, g0p_d),
            ):
                nc.sync.dma_start(out=t, in_=dd[:])

            knn_sb = big.tile([C, H], F32)  # full-batch knn kept for mean-pool

            for j in range(NTILES):
                n0 = j * NT
                # ---- loads ----
                dhi = dbuf.tile([128, K, NT], F32R, tag="dhi")
                dlo = dbuf.tile([64, K, NT], F32R, tag="dlo")
                nc.sync.dma_start(out=dhi, in_=d_r[0:128, :, n0 : n0 + NT])
                nc.sync.dma_start(out=dlo, in_=d_r[128:192, :, n0 : n0 + NT])
                hr01 = hrbuf.tile([128, NT], F32, tag="hr01")
                hr23 = hrbuf.tile([128, NT], F32, tag="hr23")
                hr4 = hrbuf.tile([64, NT], F32, tag="hr4")
                nc.sync.dma_start(out=hr01, in_=hr_d[0:128, n0 : n0 + NT])
                nc.sync.dma_start(out=hr23, in_=hr_d[128:256, n0 : n0 + NT])
                nc.sync.dma_start(out=hr4, in_=hr_d[256:320, n0 : n0 + NT])

                # ---- per-neighbor MLP ----
                pl = plp.tile([K, NT], F32)
                for k in range(K):
                    ph = php.tile([C, NT], F32, tag="ph")
                    nc.tensor.matmul(ph[:], gt_hi[:], dhi[:, k, :],
                                     start=True, stop=False)
                    nc.tensor.matmul(ph[:], gt_lo[:], dlo[:, k, :],
                                     start=False, stop=True)
                    # h_k = relu(G.d_k + g0)
                    h_k = hbuf.tile([C, NT], F32R, tag="h")
                    if k < 3:
                        nc.scalar.activation(h_k[:], ph[:], AF.Relu, bias=g0p[:])
                    else:
                        nc.vector.tensor_scalar(h_k[:], ph[:], g0p[:], 0.0,
                                                ALU.add, ALU.max)
                    # logits row k += a.d_k + v.h_k   (zero-column M=5 trick)
                    nc.tensor.matmul(pl[:], a5_hi[:, k, :], dhi[:, k, :],
                                     start=(k == 0), stop=False,
                                     skip_group_check=True)
                    nc.tensor.matmul(pl[:], a5_lo[:, k, :], dlo[:, k, :],
                                     start=False, stop=False,
                                     skip_group_check=True)
                    nc.tensor.matmul(pl[:], v5[:, k, :], h_k[:],
                                     start=False, stop=(k == K - 1),
                                     skip_group_check=True)

                # ---- softmax over k (no division: exp(l - ln(sum exp l))) ----
                e5 = smx.tile([K, NT], F32R, tag="e5")
                nc.scalar.activation(e5[:], pl[:], AF.Exp)
                ps_s = pssp.tile([1, NT], F32)
                nc.tensor.matmul(ps_s[:], ones5[:], e5[:], start=True, stop=True)
                ls = smx.tile([1, NT], F32, tag="ls")
                nc.scalar.activation(ls[:], ps_s[:], AF.Ln)
                lsb5 = smx.tile([K, NT], F32, tag="lsb5")
                ls_row = ls[0:1, :]
                nc.sync.dma_start(
                    out=lsb5,
                    in_=bass.AP(tensor=ls_row.tensor, offset=ls_row.offset,
                                ap=[ls_row.ap[0], [0, K], ls_row.ap[1]]),
                )
                wpre5 = smx.tile([K, NT], F32, tag="wpre5")
                nc.vector.tensor_sub(wpre5[:], pl[:], lsb5[:])
                wgt5 = smx.tile([K, NT], F32, tag="wgt5")
                nc.scalar.activation(wgt5[:], wpre5[:], AF.Exp)

                # ---- broadcast weights along channels (DMA replicate) ----
                wbc01 = wbcp.tile([128, NT], F32, tag="wbc01")
                wbc23 = wbcp.tile([128, NT], F32, tag="wbc23")
                wbc4 = wbcp.tile([64, NT], F32, tag="wbc4")
                for k, (dst, off) in enumerate(
                    ((wbc01, 0), (wbc01, 64), (wbc23, 0), (wbc23, 64), (wbc4, 0))
                ):
                    row = wgt5[k : k + 1, :]
                    nc.sync.dma_start(
                        out=dst[off : off + 64, :],
                        in_=bass.AP(tensor=row.tensor, offset=row.offset,
                                    ap=[row.ap[0], [0, 64], row.ap[1]]),
                    )

                # ---- weighted sum over k: DVE products + PE fold matmuls ----
                p01 = prod.tile([128, NT], F32R, tag="p01")
                p23 = prod.tile([128, NT], F32R, tag="p23")
                p4 = prod.tile([64, NT], F32R, tag="p4")
                nc.vector.tensor_mul(p01[:], hr01[:], wbc01[:])
                nc.vector.tensor_mul(p23[:], hr23[:], wbc23[:])
                nc.vector.tensor_mul(p4[:], hr4[:], wbc4[:])
                pk = pknnp.tile([C, NT], F32)
                nc.tensor.matmul(pk[:], fold2[:], p01[:], start=True, stop=False)
                nc.tensor.matmul(pk[:], fold2[:], p23[:], start=False, stop=False)
                nc.tensor.matmul(pk[:], i64[:], p4[:], start=False, stop=True)
                nc.vector.tensor_copy(knn_sb[:, n0 : n0 + NT], pk[:])
                nc.sync.dma_start(out=knn_d[:, n0 : n0 + NT],
                                  in_=knn_sb[:, n0 : n0 + NT])

            # ---- mean-pool: lr[:, n] = 0.5*(knn[:, n] + knn[:, n+4096]) ----
            for jj in range(HHALF // NT):
                n0 = jj * NT
                t = lrp.tile([C, NT], F32, tag="lrt")
                nc.vector.tensor_add(
                    t[:], knn_sb[:, n0 : n0 + NT],
                    knn_sb[:, HHALF + n0 : HHALF + n0 + NT],
                )
                lr_t = lrp.tile([C, NT], F32, tag="lrs")
                nc.vector.tensor_scalar_mul(lr_t[:], t[:], 0.5)
                nc.sync.dma_start(out=lr_d[:, n0 : n0 + NT], in_=lr_t[:])

    nc.compile()
    return nc


def _consts(w1, b1, wr1, br1, wr2, br2, w3, b3):
    w1, b1, wr1, br1, wr2, br2, w3, b3 = (
        np.asarray(t, np.float64) for t in (w1, b1, wr1, br1, wr2, br2, w3, b3)
    )
    G = wr1 @ w1                      # [64, 192]
    g0 = wr1 @ b1 + br1               # [64]
    a = (w3 @ w1)[0]                  # [192]
    v = RES_SCALE * (w3 @ wr2)[0]     # [64]

    gt = G.T.astype(np.float32).copy()         # [192, 64]
    a5_hi = np.zeros((128, K, K), np.float32)
    a5_lo = np.zeros((64, K, K), np.float32)
    v5 = np.zeros((C, K, K), np.float32)
    for k in range(K):
        a5_hi[:, k, k] = a[0:128]
        a5_lo[:, k, k] = a[128:192]
        v5[:, k, k] = v
    fold2 = np.zeros((128, C), np.float32)
    for r in range(128):
        fold2[r, r % 64] = 1.0
    return {
        "gt_hi": np.ascontiguousarray(gt[0:128]),
        "gt_lo": np.ascontiguousarray(gt[128:192]),
        "a5_hi": a5_hi,
        "a5_lo": a5_lo,
        "v5": v5,
        "ones5": np.ones((K, 1), np.float32),
        "fold2": fold2,
        "i64": np.eye(C, dtype=np.float32),
        "g0p": g0.astype(np.float32).reshape(C, 1),
    }


def kernel(knn_hr, diff_patch, w1, b1, wr1, br1, wr2, br2, w3, b3, **kw):
    knn_hr = np.ascontiguousarray(np.asarray(knn_hr, np.float32))
    diff_patch = np.ascontiguousarray(np.asarray(diff_patch, np.float32))

    if "nc" not in _CACHE:
        _CACHE["nc"] = _build_nc()
    nc = _CACHE["nc"]

    consts = _consts(w1, b1, wr1, br1, wr2, br2, w3, b3)
    in_maps = []
    for b in range(B):
        m = dict(consts)
        m["d"] = diff_patch[b]
        m["hr"] = knn_hr[b]
        in_maps.append(m)

    res = run_bass_kernel_spmd(nc, in_maps, core_ids=list(range(B)))
    knn = np.stack([res.results[b]["knn"] for b in range(B)])
    knn_lr = np.stack([res.results[b]["knn_lr"] for b in range(B)])
    return knn_lr, knn


# revision 4
# speedup vs baseline: 1.7273x; 1.7273x over previous
"""TRN2 Bass kernel for nn_GCNBlock_77927886618861 (gnn_message_passing).

Reference computation (per batch b, K=5 neighbors, H=8192 positions):
    d = diff_patch.reshape(b,5,192,H)
    x = w1@d + b1; r = relu(wr1@x + br1); r = wr2@r + br2; x = x + 0.1*r
    logits = w3@x + b3; wgt = softmax_k(logits)
    knn = sum_k knn_hr_k * wgt_k;  knn_lr = mean-pool(knn, scale=2)

The 1x1-conv MLP collapses algebraically (softmax over k is invariant to
per-position constants, so all constant terms drop):
    logits'_k = a.d_k + v.relu(G.d_k + g0)
    G = wr1@w1 [64,192], g0 = wr1@b1+br1, a = w3@w1 [192], v = 0.1*w3@wr2 [64]

The a.d_k term rides along as rows 64/65 of an extended Gt = [G; a; -a]
with zero bias there: relu(a.d) - relu(-a.d) == a.d exactly (the v-matmul
column carries +1/-1 for those rows), avoiding any magnitude shift that
would inflate float32r's relative rounding into logit error.

Sharding: one batch (of 8) per NeuronCore. Matmuls in float32r (1 cyc/col).
Softmax normalization via reciprocal_approx_fast (no Ln/table thrash, no
scalar-engine reciprocal). The k-sum of hr*wgt uses PE fold matmuls
(cross-partition adds are illegal on DVE/ACT/GPSIMD).

DMA strategy (HWDGE descriptor-gen is the scarce resource: ~15ns/descriptor,
single ring): diff_patch is rearranged on the host so each SBUF partition row
is one 10KB contiguous run; knn_hr loads in 4-tile groups (8KB runs);
knn/knn_lr written once from SBUF-resident accumulators; d-loads go through
gpsimd (SWDGE) to keep the sync ring short.
"""
import sys

import numpy as np

sys.path.insert(0, "/opt/trn_rl_repo")

import concourse.bass as bass  # noqa: E402
import concourse.bacc as bacc  # noqa: E402
import concourse.tile as tile  # noqa: E402
from concourse import mybir  # noqa: E402
from concourse.bass_utils import run_bass_kernel_spmd  # noqa: E402

F32 = mybir.dt.float32
F32R = mybir.dt.float32r
AF = mybir.ActivationFunctionType
ALU = mybir.AluOpType

B, K, C, H, DN, SCALE = 8, 5, 64, 8192, 192, 2
RES_SCALE = 0.1
NT = 512                 # positions per tile
NTILES = H // NT         # 16
GRP = 4                  # hr tiles loaded per group
HHALF = H // SCALE       # 4096
CE = C + 2               # 66 rows: G plus the [a; -a] pair

_CACHE = {}


def _bcast_row(row, parts):
    """AP reading one [1, N] SBUF row replicated across `parts` partitions."""
    return bass.AP(tensor=row.tensor, offset=row.offset,
                   ap=[row.ap[0], [0, parts], row.ap[1]])


def _build_nc():
    nc = bacc.Bacc("TRN2", target_bir_lowering=False, debug=False)

    # d is host-rearranged to [NTILES, 192, K*NT]: per tile, channel-major
    # rows of K*NT contiguous floats (10KB descriptors).
    d_d = nc.dram_tensor("d", [NTILES, DN, K * NT], F32R, kind="ExternalInput")
    hr_d = nc.dram_tensor("hr", [K * C, H], F32, kind="ExternalInput")
    gt_hi_d = nc.dram_tensor("gt_hi", [128, CE], F32R, kind="ExternalInput")
    gt_lo_d = nc.dram_tensor("gt_lo", [64, CE], F32R, kind="ExternalInput")
    v65_d = nc.dram_tensor("v65", [CE, K, K], F32R, kind="ExternalInput")
    ones5_d = nc.dram_tensor("ones5", [K, 1], F32R, kind="ExternalInput")
    bc01_d = nc.dram_tensor("bc01", [K, 128], F32R, kind="ExternalInput")
    bc23_d = nc.dram_tensor("bc23", [K, 128], F32R, kind="ExternalInput")
    bc4_d = nc.dram_tensor("bc4", [K, 64], F32R, kind="ExternalInput")
    fold2_d = nc.dram_tensor("fold2", [128, C], F32R, kind="ExternalInput")
    i64_d = nc.dram_tensor("i64", [C, C], F32R, kind="ExternalInput")
    g0p_d = nc.dram_tensor("g0p", [CE, 1], F32, kind="ExternalInput")

    knn_d = nc.dram_tensor("knn", [C, H], F32, kind="ExternalOutput")
    lr_d = nc.dram_tensor("knn_lr", [C, HHALF], F32, kind="ExternalOutput")

    with tile.TileContext(nc) as tc:
        with (
            tc.tile_pool(name="wts", bufs=1) as wts,
            tc.tile_pool(name="dbuf", bufs=2) as dbuf,
            tc.tile_pool(name="hrbuf", bufs=2) as hrbuf,
            tc.tile_pool(name="hbuf", bufs=3) as hbuf,
            tc.tile_pool(name="smx", bufs=2) as smx,
            tc.tile_pool(name="prod", bufs=2) as prod,
            tc.tile_pool(name="big", bufs=1) as bigp,
            tc.tile_pool(name="lrp", bufs=2) as lrp,
            tc.tile_pool(name="pheb", bufs=4, space="PSUM") as pheb,
            tc.tile_pool(name="pl", bufs=2, space="PSUM") as plp,
            tc.tile_pool(name="ps_s", bufs=1, space="PSUM") as pssp,
            tc.tile_pool(name="pknn", bufs=1, space="PSUM") as pknnp,
        ):
            # ---- constants (one-time) ----
            gt_hi = wts.tile([128, CE], F32R)
            gt_lo = wts.tile([64, CE], F32R)
            v65 = wts.tile([CE, K, K], F32R)
            ones5 = wts.tile([K, 1], F32R)
            bc01 = wts.tile([K, 128], F32R)
            bc23 = wts.tile([K, 128], F32R)
            bc4 = wts.tile([K, 64], F32R)
            fold2 = wts.tile([128, C], F32R)
            i64 = wts.tile([C, C], F32R)
            g0p = wts.tile([CE, 1], F32)
            for t, dd in (
                (gt_hi, gt_hi_d), (gt_lo, gt_lo_d), (v65, v65_d),
                (ones5, ones5_d), (bc01, bc01_d), (bc23, bc23_d),
                (bc4, bc4_d), (fold2, fold2_d), (i64, i64_d), (g0p, g0p_d),
            ):
                nc.sync.dma_start(out=t, in_=dd[:])

            knn_sb = bigp.tile([C, H], F32)    # full-batch knn for mean-pool
            lr_sb = bigp.tile([C, HHALF], F32)

            hr_g = [None, None, None]
            for j in range(NTILES):
                n0 = j * NT
                jg = j % GRP
                # ---- loads ----
                if jg == 0:
                    g0_ = j * NT
                    hr01 = hrbuf.tile([128, GRP, NT], F32, tag="hr01")
                    hr23 = hrbuf.tile([128, GRP, NT], F32, tag="hr23")
                    hr4 = hrbuf.tile([64, GRP, NT], F32, tag="hr4")
                    nc.sync.dma_start(
                        out=hr01, in_=hr_d[0:128, g0_ : g0_ + GRP * NT])
                    nc.sync.dma_start(
                        out=hr23, in_=hr_d[128:256, g0_ : g0_ + GRP * NT])
                    nc.sync.dma_start(
                        out=hr4, in_=hr_d[256:320, g0_ : g0_ + GRP * NT])
                    hr_g = [hr01, hr23, hr4]
                dhi = dbuf.tile([128, K, NT], F32R, tag="dhi")
                dlo = dbuf.tile([64, K, NT], F32R, tag="dlo")
                nc.gpsimd.dma_start(
                    out=dhi[:].rearrange("c k n -> c (k n)"),
                    in_=d_d[j, 0:128, :])
                nc.gpsimd.dma_start(
                    out=dlo[:].rearrange("c k n -> c (k n)"),
                    in_=d_d[j, 128:192, :])

                # ---- per-neighbor MLP + logits ----
                pl = plp.tile([K, NT], F32)
                for k in range(K):
                    ph = pheb.tile([CE, NT], F32, tag="psum_big")
                    nc.tensor.matmul(ph[:], gt_hi[:], dhi[:, k, :],
                                     start=True, stop=False)
                    nc.tensor.matmul(ph[:], gt_lo[:], dlo[:, k, :],
                                     start=False, stop=True)
                    # h = relu([G;a;-a].d_k + [g0;0;0])
                    h_k = hbuf.tile([CE, NT], F32R, tag="h")
                    if k < 3:
                        nc.scalar.activation(h_k[:], ph[:], AF.Relu, bias=g0p[:])
                    else:
                        nc.vector.tensor_scalar(h_k[:], ph[:], g0p[:], 0.0,
                                                ALU.add, ALU.max)
                    # logits row k += v.h_k + relu(a.d_k) - relu(-a.d_k)
                    nc.tensor.matmul(pl[:], v65[:, k, :], h_k[:],
                                     start=(k == 0), stop=(k == K - 1),
                                     skip_group_check=True)

                # ---- softmax over k: e / sum(e) ----
                e5 = smx.tile([K, NT], F32R, tag="e5")
                nc.scalar.activation(e5[:], pl[:], AF.Exp)
                ps_s = pssp.tile([1, NT], F32)
                nc.tensor.matmul(ps_s[:], ones5[:], e5[:], start=True, stop=True)
                r_sb = smx.tile([1, NT], F32, tag="r_sb")
                nc.vector.reciprocal_approx_fast(out=r_sb[:], in_=ps_s[:])
                rbc5 = smx.tile([K, NT], F32, tag="rbc5")
                nc.sync.dma_start(out=rbc5, in_=_bcast_row(r_sb[0:1, :], K))
                wgt5 = smx.tile([K, NT], F32R, tag="wgt5")
                nc.gpsimd.tensor_tensor(wgt5[:], e5[:].bitcast(F32), rbc5[:],
                                        ALU.mult)

                # ---- broadcast weights along channels (PE matmuls) ----
                eb01 = pheb.tile([128, NT], F32, tag="psum_big")
                eb23 = pheb.tile([128, NT], F32, tag="psum_big")
                eb4 = pheb.tile([64, NT], F32, tag="psum_big")
                nc.tensor.matmul(eb01[:], bc01[:], wgt5[:], start=True, stop=True)
                nc.tensor.matmul(eb23[:], bc23[:], wgt5[:], start=True, stop=True)
                nc.tensor.matmul(eb4[:], bc4[:], wgt5[:], start=True, stop=True)

                # ---- weighted products + PE fold over k ----
                p01 = prod.tile([128, NT], F32R, tag="p01")
                p23 = prod.tile([128, NT], F32R, tag="p23")
                p4 = prod.tile([64, NT], F32R, tag="p4")
                nc.vector.tensor_mul(p01[:], hr_g[0][:, jg, :], eb01[:])
                nc.vector.tensor_mul(p23[:], hr_g[1][:, jg, :], eb23[:])
                nc.vector.tensor_mul(p4[:], hr_g[2][:, jg, :], eb4[:])
                pk = pknnp.tile([C, NT], F32)
                nc.tensor.matmul(pk[:], fold2[:], p01[:], start=True, stop=False)
                nc.tensor.matmul(pk[:], fold2[:], p23[:], start=False, stop=False)
                nc.tensor.matmul(pk[:], i64[:], p4[:], start=False, stop=True)
                nc.scalar.activation(knn_sb[:, n0 : n0 + NT], pk[:], AF.Copy)

            # ---- mean-pool: lr[:, n] = 0.5*(knn[:, n] + knn[:, n+4096]) ----
            for jj in range(HHALF // NT):
                n0 = jj * NT
                t = lrp.tile([C, NT], F32, tag="lrt")
                nc.gpsimd.tensor_tensor(
                    t[:], knn_sb[:, n0 : n0 + NT],
                    knn_sb[:, HHALF + n0 : HHALF + n0 + NT], ALU.add,
                )
                nc.gpsimd.tensor_scalar_mul(
                    lr_sb[:, n0 : n0 + NT], t[:], 0.5)

            nc.sync.dma_start(out=knn_d[:], in_=knn_sb[:])
            nc.sync.dma_start(out=lr_d[:], in_=lr_sb[:])

    nc.compile()
    return nc


def _consts(w1, b1, wr1, br1, wr2, br2, w3, b3):
    w1, b1, wr1, br1, wr2, br2, w3, b3 = (
        np.asarray(t, np.float64) for t in (w1, b1, wr1, br1, wr2, br2, w3, b3)
    )
    G = wr1 @ w1                      # [64, 192]
    g0 = wr1 @ b1 + br1               # [64]
    a = (w3 @ w1)[0]                  # [192]
    v = RES_SCALE * (w3 @ wr2)[0]     # [64]

    Ge = np.concatenate([G, a[None, :], -a[None, :]], axis=0)  # [66, 192]
    gte = Ge.T.astype(np.float32).copy()            # [192, 66]
    v65 = np.zeros((CE, K, K), np.float32)
    for k in range(K):
        v65[0:C, k, k] = v
        v65[C, k, k] = 1.0
        v65[C + 1, k, k] = -1.0
    bc01 = np.zeros((K, 128), np.float32)
    bc01[0, 0:64] = 1.0
    bc01[1, 64:128] = 1.0
    bc23 = np.zeros((K, 128), np.float32)
    bc23[2, 0:64] = 1.0
    bc23[3, 64:128] = 1.0
    bc4 = np.zeros((K, 64), np.float32)
    bc4[4, :] = 1.0
    fold2 = np.zeros((128, C), np.float32)
    for r in range(128):
        fold2[r, r % 64] = 1.0
    g0p = np.concatenate([g0, [0.0, 0.0]]).astype(np.float32).reshape(CE, 1)
    return {
        "gt_hi": np.ascontiguousarray(gte[0:128]),
        "gt_lo": np.ascontiguousarray(gte[128:192]),
        "v65": v65,
        "ones5": np.ones((K, 1), np.float32),
        "bc01": bc01,
        "bc23": bc23,
        "bc4": bc4,
        "fold2": fold2,
        "i64": np.eye(C, dtype=np.float32),
        "g0p": g0p,
    }


def kernel(knn_hr, diff_patch, w1, b1, wr1, br1, wr2, br2, w3, b3, **kw):
    knn_hr = np.ascontiguousarray(np.asarray(knn_hr, np.float32))
    diff_patch = np.asarray(diff_patch, np.float32)

    # [B, 5*192, H] -> [B, NTILES, 192, K*NT]: per tile, channel rows hold
    # all 5 neighbors' NT positions contiguously.
    d_re = np.ascontiguousarray(
        diff_patch.reshape(B, K, DN, NTILES, NT).transpose(0, 3, 2, 1, 4)
    ).reshape(B, NTILES, DN, K * NT)

    if "nc" not in _CACHE:
        _CACHE["nc"] = _build_nc()
    nc = _CACHE["nc"]

    consts = _consts(w1, b1, wr1, br1, wr2, br2, w3, b3)
    in_maps = []
    for b in range(B):
        m = dict(consts)
        m["d"] = d_re[b]
        m["hr"] = knn_hr[b]
        in_maps.append(m)

    res = run_bass_kernel_spmd(nc, in_maps, core_ids=list(range(B)))
    knn = np.stack([res.results[b]["knn"] for b in range(B)])
    knn_lr = np.stack([res.results[b]["knn_lr"] for b in range(B)])
    return knn_lr, knn


# revision 5
# speedup vs baseline: 2.3976x; 1.3881x over previous
"""TRN2 Bass kernel for nn_GCNBlock_77927886618861 (gnn_message_passing).

Reference computation (per batch b, K=5 neighbors, H=8192 positions):
    d = diff_patch.reshape(b,5,192,H)
    x = w1@d + b1; r = relu(wr1@x + br1); r = wr2@r + br2; x = x + 0.1*r
    logits = w3@x + b3; wgt = softmax_k(logits)
    knn = sum_k knn_hr_k * wgt_k;  knn_lr = mean-pool(knn, scale=2)

The 1x1-conv MLP collapses algebraically (softmax over k is invariant to
per-position constants, so all constant terms drop):
    logits'_k = a.d_k + v.relu(G.d_k + g0)
    G = wr1@w1 [64,192], g0 = wr1@b1+br1, a = w3@w1 [192], v = 0.1*w3@wr2 [64]

The a.d_k term rides along as rows 64/65 of an extended Gt = [G; a; -a]
with zero bias there: relu(a.d) - relu(-a.d) == a.d exactly (the v-matmul
column carries +1/-1 for those rows), avoiding any magnitude shift that
would inflate float32r's relative rounding into logit error.

Sharding: one batch (of 8) per NeuronCore. Matmuls in float32r (1 cyc/col).
Softmax normalization via reciprocal_approx_fast (no Ln/table thrash, no
scalar-engine reciprocal). The k-sum of hr*wgt uses PE fold matmuls
(cross-partition adds are illegal on DVE/ACT/GPSIMD).

DMA strategy (HWDGE descriptor-gen is the scarce resource: ~15ns/descriptor,
single ring): diff_patch is rearranged on the host so each SBUF partition row
is one 10KB contiguous run; knn_hr loads in 4-tile groups (8KB runs);
knn/knn_lr written once from SBUF-resident accumulators; d-loads go through
gpsimd (SWDGE) to keep the sync ring short.
"""
import sys

import numpy as np

sys.path.insert(0, "/opt/trn_rl_repo")

import concourse.bass as bass  # noqa: E402
import concourse.bacc as bacc  # noqa: E402
import concourse.tile as tile  # noqa: E402
from concourse import mybir  # noqa: E402
from concourse.bass_utils import run_bass_kernel_spmd  # noqa: E402

F32 = mybir.dt.float32
F32R = mybir.dt.float32r
AF = mybir.ActivationFunctionType
ALU = mybir.AluOpType

B, K, C, H, DN, SCALE = 8, 5, 64, 8192, 192, 2
RES_SCALE = 0.1
NT = 512                 # positions per tile
NTILES = H // NT         # 16
GRP = 4                  # hr tiles loaded per group
HHALF = H // SCALE       # 4096
CE = C + 2               # 66 rows: G plus the [a; -a] pair

_CACHE = {}


def _bcast_row(row, parts):
    """AP reading one [1, N] SBUF row replicated across `parts` partitions."""
    return bass.AP(tensor=row.tensor, offset=row.offset,
                   ap=[row.ap[0], [0, parts], row.ap[1]])


def _build_nc():
    nc = bacc.Bacc("TRN2", target_bir_lowering=False, debug=False)

    # d is host-rearranged to [NTILES, 192, K*NT]: per tile, channel-major
    # rows of K*NT contiguous floats (10KB descriptors).
    d_d = nc.dram_tensor("d", [NTILES, DN, K * NT], F32R, kind="ExternalInput")
    hr_d = nc.dram_tensor("hr", [K * C, H], F32, kind="ExternalInput")
    gt_hi_d = nc.dram_tensor("gt_hi", [128, CE], F32R, kind="ExternalInput")
    gt_lo_d = nc.dram_tensor("gt_lo", [64, CE], F32R, kind="ExternalInput")
    v65_d = nc.dram_tensor("v65", [CE, K, K], F32R, kind="ExternalInput")
    ones5_d = nc.dram_tensor("ones5", [K, 1], F32R, kind="ExternalInput")
    bc01_d = nc.dram_tensor("bc01", [K, 128], F32R, kind="ExternalInput")
    bc23_d = nc.dram_tensor("bc23", [K, 128], F32R, kind="ExternalInput")
    bc4_d = nc.dram_tensor("bc4", [K, 64], F32R, kind="ExternalInput")
    fold2_d = nc.dram_tensor("fold2", [128, C], F32R, kind="ExternalInput")
    i64_d = nc.dram_tensor("i64", [C, C], F32R, kind="ExternalInput")
    g0p_d = nc.dram_tensor("g0p", [CE, 1], F32, kind="ExternalInput")

    knn_d = nc.dram_tensor("knn", [C, H], F32, kind="ExternalOutput")
    lr_d = nc.dram_tensor("knn_lr", [C, HHALF], F32, kind="ExternalOutput")

    with tile.TileContext(nc) as tc:
        with (
            tc.tile_pool(name="wts", bufs=1) as wts,
            tc.tile_pool(name="dbuf", bufs=3) as dbuf,
            tc.tile_pool(name="hrbuf", bufs=2) as hrbuf,
            tc.tile_pool(name="hbuf", bufs=3) as hbuf,
            tc.tile_pool(name="smx", bufs=2) as smx,
            tc.tile_pool(name="prod", bufs=3) as prod,
            tc.tile_pool(name="big", bufs=1) as bigp,
            tc.tile_pool(name="lrp", bufs=2) as lrp,
            tc.tile_pool(name="pheb", bufs=3, space="PSUM") as pheb,
            tc.tile_pool(name="pl", bufs=2, space="PSUM") as plp,
            tc.tile_pool(name="ps_s", bufs=1, space="PSUM") as pssp,
            tc.tile_pool(name="pknn", bufs=2, space="PSUM") as pknnp,
        ):
            # ---- constants (one-time) ----
            gt_hi = wts.tile([128, CE], F32R)
            gt_lo = wts.tile([64, CE], F32R)
            v65 = wts.tile([CE, K, K], F32R)
            ones5 = wts.tile([K, 1], F32R)
            bc01 = wts.tile([K, 128], F32R)
            bc23 = wts.tile([K, 128], F32R)
            bc4 = wts.tile([K, 64], F32R)
            fold2 = wts.tile([128, C], F32R)
            i64 = wts.tile([C, C], F32R)
            g0p = wts.tile([CE, 1], F32)
            for t, dd in (
                (gt_hi, gt_hi_d), (gt_lo, gt_lo_d), (v65, v65_d),
                (ones5, ones5_d), (bc01, bc01_d), (bc23, bc23_d),
                (bc4, bc4_d), (fold2, fold2_d), (i64, i64_d), (g0p, g0p_d),
            ):
                nc.sync.dma_start(out=t, in_=dd[:])

            knn_sb = bigp.tile([C, H], F32)    # full-batch knn for mean-pool
            lr_sb = bigp.tile([C, HHALF], F32)

            hr_g = [None, None, None]
            for j in range(NTILES):
                n0 = j * NT
                jg = j % GRP
                # ---- loads ----
                if jg == 0:
                    g0_ = j * NT
                    hr01 = hrbuf.tile([128, GRP, NT], F32, tag="hr01")
                    hr23 = hrbuf.tile([128, GRP, NT], F32, tag="hr23")
                    hr4 = hrbuf.tile([64, GRP, NT], F32, tag="hr4")
                    nc.sync.dma_start(
                        out=hr01, in_=hr_d[0:128, g0_ : g0_ + GRP * NT])
                    nc.sync.dma_start(
                        out=hr23, in_=hr_d[128:256, g0_ : g0_ + GRP * NT])
                    nc.sync.dma_start(
                        out=hr4, in_=hr_d[256:320, g0_ : g0_ + GRP * NT])
                    hr_g = [hr01, hr23, hr4]
                dhi = dbuf.tile([128, K, NT], F32R, tag="dhi")
                dlo = dbuf.tile([64, K, NT], F32R, tag="dlo")
                nc.gpsimd.dma_start(
                    out=dhi[:].rearrange("c k n -> c (k n)"),
                    in_=d_d[j, 0:128, :])
                nc.gpsimd.dma_start(
                    out=dlo[:].rearrange("c k n -> c (k n)"),
                    in_=d_d[j, 128:192, :])

                # ---- per-neighbor MLP + logits ----
                pl = plp.tile([K, NT], F32)
                for k in range(K):
                    ph = pheb.tile([CE, NT], F32, tag="psum_big")
                    nc.tensor.matmul(ph[:], gt_hi[:], dhi[:, k, :],
                                     start=True, stop=False)
                    nc.tensor.matmul(ph[:], gt_lo[:], dlo[:, k, :],
                                     start=False, stop=True)
                    # h = relu([G;a;-a].d_k + [g0;0;0])
                    h_k = hbuf.tile([CE, NT], F32R, tag="h")
                    nc.scalar.activation(h_k[:], ph[:], AF.Relu, bias=g0p[:])
                    # logits row k += v.h_k + relu(a.d_k) - relu(-a.d_k)
                    nc.tensor.matmul(pl[:], v65[:, k, :], h_k[:],
                                     start=(k == 0), stop=(k == K - 1),
                                     skip_group_check=True)

                # ---- softmax over k: e / sum(e) ----
                e5 = smx.tile([K, NT], F32R, tag="e5")
                nc.scalar.activation(e5[:], pl[:], AF.Exp)
                ps_s = pssp.tile([1, NT], F32)
                nc.tensor.matmul(ps_s[:], ones5[:], e5[:], start=True, stop=True)
                r_sb = smx.tile([1, NT], F32, tag="r_sb")
                nc.vector.reciprocal_approx_fast(out=r_sb[:], in_=ps_s[:])
                rbc64 = smx.tile([C, NT], F32, tag="rbc64")
                nc.sync.dma_start(out=rbc64, in_=_bcast_row(r_sb[0:1, :], C))

                # ---- broadcast (unnormalized) weights along channels ----
                eb01 = pheb.tile([128, NT], F32, tag="psum_big")
                eb23 = pheb.tile([128, NT], F32, tag="psum_big")
                eb4 = pheb.tile([64, NT], F32, tag="psum_big")
                nc.tensor.matmul(eb01[:], bc01[:], e5[:], start=True, stop=True)
                nc.tensor.matmul(eb23[:], bc23[:], e5[:], start=True, stop=True)
                nc.tensor.matmul(eb4[:], bc4[:], e5[:], start=True, stop=True)

                # ---- weighted products + PE fold over k ----
                p01 = prod.tile([128, NT], F32R, tag="p01")
                p23 = prod.tile([128, NT], F32R, tag="p23")
                p4 = prod.tile([64, NT], F32R, tag="p4")
                nc.vector.tensor_mul(p01[:], hr_g[0][:, jg, :], eb01[:])
                nc.vector.tensor_mul(p23[:], hr_g[1][:, jg, :], eb23[:])
                nc.vector.tensor_mul(p4[:], hr_g[2][:, jg, :], eb4[:])
                pk = pknnp.tile([C, NT], F32)
                nc.tensor.matmul(pk[:], fold2[:], p01[:], start=True, stop=False)
                nc.tensor.matmul(pk[:], fold2[:], p23[:], start=False, stop=False)
                nc.tensor.matmul(pk[:], i64[:], p4[:], start=False, stop=True)
                # knn = (sum_k hr_k * e_k) / sum_k e_k
                nc.vector.tensor_mul(knn_sb[:, n0 : n0 + NT], pk[:], rbc64[:])

                # ---- interleaved mean-pool once the paired tile exists ----
                if j >= HHALF // NT:
                    m0 = n0 - HHALF
                    t = lrp.tile([C, NT], F32, tag="lrt")
                    nc.vector.tensor_add(
                        t[:], knn_sb[:, m0 : m0 + NT],
                        knn_sb[:, n0 : n0 + NT],
                    )
                    nc.scalar.activation(lr_sb[:, m0 : m0 + NT], t[:],
                                         AF.Copy, scale=0.5)

            nc.sync.dma_start(out=knn_d[:], in_=knn_sb[:])
            nc.sync.dma_start(out=lr_d[:], in_=lr_sb[:])

    nc.compile()
    return nc


def _consts(w1, b1, wr1, br1, wr2, br2, w3, b3):
    w1, b1, wr1, br1, wr2, br2, w3, b3 = (
        np.asarray(t, np.float64) for t in (w1, b1, wr1, br1, wr2, br2, w3, b3)
    )
    G = wr1 @ w1                      # [64, 192]
    g0 = wr1 @ b1 + br1               # [64]
    a = (w3 @ w1)[0]                  # [192]
    v = RES_SCALE * (w3 @ wr2)[0]     # [64]

    Ge = np.concatenate([G, a[None, :], -a[None, :]], axis=0)  # [66, 192]
    gte = Ge.T.astype(np.float32).copy()            # [192, 66]
    v65 = np.zeros((CE, K, K), np.float32)
    for k in range(K):
        v65[0:C, k, k] = v
        v65[C, k, k] = 1.0
        v65[C + 1, k, k] = -1.0
    bc01 = np.zeros((K, 128), np.float32)
    bc01[0, 0:64] = 1.0
    bc01[1, 64:128] = 1.0
    bc23 = np.zeros((K, 128), np.float32)
    bc23[2, 0:64] = 1.0
    bc23[3, 64:128] = 1.0
    bc4 = np.zeros((K, 64), np.float32)
    bc4[4, :] = 1.0
    fold2 = np.zeros((128, C), np.float32)
    for r in range(128):
        fold2[r, r % 64] = 1.0
    g0p = np.concatenate([g0, [0.0, 0.0]]).astype(np.float32).reshape(CE, 1)
    return {
        "gt_hi": np.ascontiguousarray(gte[0:128]),
        "gt_lo": np.ascontiguousarray(gte[128:192]),
        "v65": v65,
        "ones5": np.ones((K, 1), np.float32),
        "bc01": bc01,
        "bc23": bc23,
        "bc4": bc4,
        "fold2": fold2,
        "i64": np.eye(C, dtype=np.float32),
        "g0p": g0p,
    }


def kernel(knn_hr, diff_patch, w1, b1, wr1, br1, wr2, br2, w3, b3, **kw):
    knn_hr = np.ascontiguousarray(np.asarray(knn_hr, np.float32))
    diff_patch = np.asarray(diff_patch, np.float32)

    # [B, 5*192, H] -> [B, NTILES, 192, K*NT]: per tile, channel rows hold
    # all 5 neighbors' NT positions contiguously.
    d_re = np.ascontiguousarray(
        diff_patch.reshape(B, K, DN, NTILES, NT).transpose(0, 3, 2, 1, 4)
    ).reshape(B, NTILES, DN, K * NT)

    if "nc" not in _CACHE:
        _CACHE["nc"] = _build_nc()
    nc = _CACHE["nc"]

    consts = _consts(w1, b1, wr1, br1, wr2, br2, w3, b3)
    in_maps = []
    for b in range(B):
        m = dict(consts)
        m["d"] = d_re[b]
        m["hr"] = knn_hr[b]
        in_maps.append(m)

    res = run_bass_kernel_spmd(nc, in_maps, core_ids=list(range(B)))
    knn = np.stack([res.results[b]["knn"] for b in range(B)])
    knn_lr = np.stack([res.results[b]["knn_lr"] for b in range(B)])
    return knn_lr, knn
